# revision 14
# baseline (speedup 1.0000x reference)
"""GNN message-passing kernel for 8 TRN2 NeuronCores (Bass/Tile, SPMD).

Takes the FULL inputs of nn_Base_40793599378196, returns the FULL [512, 130]
output.  Nodes/edges are sharded by graph (core c owns nodes [c*8192,
(c+1)*8192)); weights are replicated.

Design (2.20 ms on HW, vs 2.88 ms for the y-space baseline):
- Aggregation commuted into RAW feature space: segment_sum(h[src]) @ wn ==
  segment_sum((h @ wn)[src]), so the gather table for layer l is h_{l-1}
  itself and the per-layer y=h@wn pre-transform disappears.  Layer 0
  gathers from a pair-packed x table that is a kernel INPUT (no AllGather,
  no y0 build); layers 1-2 gather pair rows of the AllGathered node-major
  h table (pair index fits int16: no lo/hi split).
- dma_gather descriptor generation (~9 ns/edge, flat in call/elem size) is
  the hard bottleneck, so slot count is minimized: 128-slot chunks uniform
  in (512-node dst window, src parity) give 545 chunks/layer (vs 640 at
  128-node windows); scatter-add = one-hot matmuls (is_equal masks built 8
  chunks per DVE op; iota/dstrel in fp16 so values up to 512 stay exact)
  accumulating [feat, 512] per window in one PSUM bank; one scalar-engine
  copy per window into an SBUF g buffer.
- The combined transform h_raw = wr^T h + wn^T g and BN statistics run per
  512-column tile interleaved into the gather-call loop (one tile per
  call), hiding under the next dma_gather.  BN mean/var AllReduce (1 KB);
  conv bias folds into the BN shift; relu(scale*h+shift) on the scalar
  engine; h is PE-transposed to node-major only at layer boundaries for
  the next AllGather (16 MB, ~69 us).
- Graph pooling = free-dim window reduction; graph head + 128 per-node
  head MLPs run on the 64 local graphs.

Compute dtype: bf16 operands, fp32 PSUM/statistics (rel err ~1.2e-2).
"""

import numpy as np
import ml_dtypes

import concourse.bacc as bacc
import concourse.tile as tile
import concourse.mybir as mybir
from concourse.bass_utils import run_bass_kernel_spmd
from concourse.instruction_name_ordered_set import InstructionNameOrderedSet

F32 = mybir.dt.float32
BF16 = mybir.dt.bfloat16
I16 = mybir.dt.int16
FP16 = mybir.dt.float16
AF = mybir.ActivationFunctionType
OP = mybir.AluOpType

NBF = ml_dtypes.bfloat16

N = 65536
E = 524288
H = 128
B = 512
NPG = 128
NC = 8
NPC = N // NC      # 8192 nodes per core
W = 512            # dst scatter-window width
NW = NPC // W      # 32 scatter windows per core
NT = 64            # 128-node transpose tiles per core
GPC = B // NC      # 64 graphs per core
EPS = 1e-5

WPC = 1            # windows per gather call

SP = False         # dma_gather single_packet
QUEUES = [0]       # round-robin queue assignment for gather calls
PREP = False       # use prepare_only + trigger_dma for gathers


# ----------------------------------------------------------------- host prep

def _build_plan(edge_index, ng):
    """Chunk plan with chunks uniform in (dst window, src % ng).

    Global-uniform chunk structure (max over cores) so the SPMD program is
    identical on every core.  Returns:
      chunk_win  [C]   window of each chunk
      chunk_grp  [C]   src%ng group of each chunk (lhsT column slice)
      calls      list of (chunk_lo, chunk_hi) per gather call
      idx_all    [NC, C, 128] int16   (src // ng, 0-padded)
      dr_all     [NC, C, 128] float32 (dst % W, W-padded)
    """
    src = edge_index[0].astype(np.int64)
    dst = edge_index[1].astype(np.int64)
    core = dst // NPC
    win = (dst % NPC) // W
    grp = src % ng
    gidx = src // ng
    drel = dst % W

    key = (core * NW + win) * ng + grp
    order = np.argsort(key, kind="stable")
    key_s = key[order]
    gidx_s = gidx[order]
    drel_s = drel[order]
    counts = np.bincount(key_s, minlength=NC * NW * ng).reshape(NC, NW, ng)
    nch = np.maximum(1, -(-counts.max(axis=0) // 128))  # [NW, ng] global max

    starts = np.zeros(NC * NW * ng + 1, dtype=np.int64)
    np.cumsum(counts.reshape(-1), out=starts[1:])

    chunk_win, chunk_grp = [], []
    for w in range(NW):
        for g in range(ng):
            for _ in range(nch[w, g]):
                chunk_win.append(w)
                chunk_grp.append(g)
    C = len(chunk_win)
    chunk_win = np.array(chunk_win)
    chunk_grp = np.array(chunk_grp)

    idx_all = np.zeros((NC, C, 128), np.int16)
    dr_all = np.full((NC, C, 128), float(W), np.float32)
    for c in range(NC):
        ci = 0
        for w in range(NW):
            for g in range(ng):
                k = (c * NW + w) * ng + g
                s0, s1 = starts[k], starts[k + 1]
                n = s1 - s0
                for j in range(nch[w, g]):
                    a = s0 + j * 128
                    b = min(s0 + (j + 1) * 128, s1)
                    if b > a:
                        idx_all[c, ci, : b - a] = gidx_s[a:b].astype(np.int16)
                        dr_all[c, ci, : b - a] = drel_s[a:b]
                    ci += 1
    assert ci == C

    # call boundaries: split at window boundaries every WPC windows
    calls = []
    lo = 0
    for wstart in range(0, NW, WPC):
        wend = wstart + WPC
        hi = int(np.searchsorted(chunk_win, wend, side="left"))
        calls.append((lo, hi))
        lo = hi
    assert lo == C
    return chunk_win, chunk_grp, calls, idx_all, dr_all


def _wrap_calls(idx_core, calls):
    """Wrap each call's flat idx block into the dma_gather 16-partition
    layout, replicated over the 8 groups; concatenate across calls."""
    blocks = []
    for lo, hi in calls:
        flat = idx_core[lo:hi].reshape(-1)       # [nch*128]
        a = flat.reshape(-1, 16).T               # [16, nch*8]
        blocks.append(np.tile(a, (8, 1)))
    return np.concatenate(blocks, axis=1).copy()


# -------------------------------------------------------------- device build

def _build(nc, plan12):
    cw12, cg12, calls12, _, _ = plan12
    C12 = len(cw12)

    def din(name, shape, dt):
        return nc.dram_tensor(name, shape, dt, kind="ExternalInput").ap()

    xq = din("xq", [N // 2, 128], BF16)            # pair-packed x table (cols 0-63 used)
    xloc = din("xloc", [32, NPC], BF16)            # local x feature-major
    idx12 = din("idx12", [128, C12 * 8], I16)
    dr12 = din("dr12", [128, C12], F32)
    iota = din("iota", [128, W], FP16)
    ident = din("ident", [128, 128], BF16)
    wn0 = din("wn0", [32, 128], BF16)
    wr0 = din("wr0", [32, 128], BF16)
    wn12 = din("wn12", [2, 128, 128], BF16)
    wr12 = din("wr12", [2, 128, 128], BF16)
    cb = din("cb", [128, 3], F32)
    bng = din("bng", [128, 3], F32)
    bnb = din("bnb", [128, 3], F32)
    gsw1 = din("gsw1", [128, 128], BF16)
    gsw2 = din("gsw2", [128, 128], BF16)
    ghw1 = din("ghw1", [128, 128], BF16)
    ghw2 = din("ghw2", [128, 64], BF16)
    ghw3 = din("ghw3", [64, 2], BF16)
    gsb1 = din("gsb1", [128, 1], F32)
    gsb2 = din("gsb2", [128, 1], F32)
    ghb1 = din("ghb1", [128, 1], F32)
    ghb2 = din("ghb2", [64, 1], F32)
    ghb3 = din("ghb3", [2, 1], F32)
    nhw1 = din("nhw1", [128, 128 * 128], BF16)
    nhw2 = din("nhw2", [128, 128 * 64], BF16)
    nhw3 = din("nhw3", [64, 128], BF16)
    nhb1 = din("nhb1", [128, 128], F32)
    nhb2 = din("nhb2", [64, 128], F32)
    nhb3 = din("nhb3", [1, 128], F32)

    outg = nc.dram_tensor("outg", [2, GPC], F32, kind="ExternalOutput").ap()
    outn = nc.dram_tensor("outn", [128, GPC], F32, kind="ExternalOutput").ap()

    hN = nc.dram_tensor("hN", [NPC, 128], BF16).ap()
    hfull = nc.dram_tensor("hfull", [N, 128], BF16, addr_space="Shared").ap()
    bn_in = [nc.dram_tensor(f"bn_in{l}", [128, 2], F32).ap() for l in range(3)]
    bn_out = [
        nc.dram_tensor(f"bn_out{l}", [128, 2], F32, addr_space="Shared").ap()
        for l in range(3)
    ]
    rg = [list(range(NC))]

    with tile.TileContext(nc) as tc:
        dma_sem = nc.alloc_semaphore("gdma")
        with (
            tc.tile_pool(name="persist", bufs=1) as pp,
            tc.tile_pool(name="small", bufs=2) as sp,
        ):
            idx12_sb = pp.tile([128, C12 * 8], I16)
            nc.sync.dma_start(out=idx12_sb[:], in_=idx12)
            xloc_sb = pp.tile([32, NPC], BF16)
            nc.sync.dma_start(out=xloc_sb[:], in_=xloc)
            dr12_sb = pp.tile([128, C12], F32)
            nc.sync.dma_start(out=dr12_sb[:], in_=dr12)
            iota_sb = pp.tile([128, W], FP16)
            nc.sync.dma_start(out=iota_sb[:], in_=iota)
            ident_sb = pp.tile([128, 128], BF16)
            nc.sync.dma_start(out=ident_sb[:], in_=ident)

            wn0_sb = pp.tile([32, 128], BF16)
            nc.sync.dma_start(out=wn0_sb[:], in_=wn0)
            wr0_sb = pp.tile([32, 128], BF16)
            nc.sync.dma_start(out=wr0_sb[:], in_=wr0)
            wn12_sb = pp.tile([128, 2, 128], BF16)
            wr12_sb = pp.tile([128, 2, 128], BF16)
            for l in range(2):
                nc.sync.dma_start(out=wn12_sb[:, l, :], in_=wn12[l])
                nc.sync.dma_start(out=wr12_sb[:, l, :], in_=wr12[l])
            cb_sb = pp.tile([128, 3], F32)
            nc.sync.dma_start(out=cb_sb[:], in_=cb)
            bng_sb = pp.tile([128, 3], F32)
            nc.sync.dma_start(out=bng_sb[:], in_=bng)
            bnb_sb = pp.tile([128, 3], F32)
            nc.sync.dma_start(out=bnb_sb[:], in_=bnb)

            h_bf = pp.tile([128, NPC], BF16)     # current layer input h
            h_raw = pp.tile([128, NPC], BF16)    # pre-BN output
            s1 = pp.tile([128, 16], F32)
            s2 = pp.tile([128, 16], F32)

            pair_tbl = hfull.rearrange("(a b) f -> a (b f)", b=2)

            with (
                tc.tile_pool(name="msg", bufs=5) as mp,
                tc.tile_pool(name="oh", bufs=6) as ohp,
                tc.tile_pool(name="gw", bufs=3) as gwp,
                tc.tile_pool(name="sq", bufs=2) as qp,
                tc.tile_pool(name="psW", bufs=2, space="PSUM") as psW,
                tc.tile_pool(name="psC", bufs=2, space="PSUM") as psC,
                tc.tile_pool(name="psT", bufs=2, space="PSUM") as psT,
            ):
                # SWDGE prep/trigger bookkeeping: preps and triggers each
                # form a no-sync chain (ring FIFO order); prep g additionally
                # chains behind trigger g-AHEAD to bound descriptor-ring use.
                AHEAD = 4
                preps: list = []
                trigs: list = []

                def chain(inst, prev):
                    if prev is not None:
                        s = InstructionNameOrderedSet()
                        s.add(prev.ins.name)
                        inst.ins.add_nosync_dependencies_from(s)

                for l in range(3):
                    cwin, cgrp, calls = cw12, cg12, calls12
                    idx_sb, dr_sb = idx12_sb, dr12_sb
                    if l == 0:
                        tbl = xq
                        FP = 32              # feat per group slice
                        hin = xloc_sb
                        wr_l, wn_l = wr0_sb[:], wn0_sb[:]
                    else:
                        tbl = pair_tbl
                        FP = 128
                        hin = h_bf
                        wr_l = wr12_sb[:, l - 1, :]
                        wn_l = wn12_sb[:, l - 1, :]

                    C = len(cwin)
                    MC = max(hi - lo for lo, hi in calls)
                    wps = None
                    ioff = 0
                    ncall = 0
                    for lo, hi in calls:
                        nch = hi - lo
                        ni = nch * 128
                        msg = mp.tile([128, MC, 256], BF16, tag="msg")
                        if FP == 128:
                            mv = msg[:]
                        else:
                            mv = msg[:].rearrange("p a (x b) -> p (a x) b", x=2)
                        gout = mv[:, :nch, :]
                        if PREP:
                            prep = nc.gpsimd.dma_gather(
                                gout,
                                tbl,
                                idx_sb[:, ioff : ioff + ni // 16],
                                ni,
                                ni,
                                256 if FP == 128 else 128,
                                single_packet=SP,
                                queue_num=0,
                                prepare_only=True,
                                sem=dma_sem,
                            )
                            chain(prep, preps[-1] if preps else None)
                            if len(trigs) >= AHEAD:
                                chain(prep, trigs[len(preps) - AHEAD])
                            preps.append(prep)
                            trig = nc.gpsimd.trigger_dma(count=None)
                            chain(trig, trigs[-1] if trigs else None)
                            trigs.append(trig)
                            # Tile's DMASW bookkeeping under-syncs consumers
                            # of a prepared gather; gate the first msg reader
                            # on the DMA-completion sem explicitly (16 incs
                            # per call, in trigger order).
                            dma_target = 16 * len(trigs)
                        else:
                            nc.gpsimd.dma_gather(
                                gout,
                                tbl,
                                idx_sb[:, ioff : ioff + ni // 16],
                                ni,
                                ni,
                                256 if FP == 128 else 128,
                                single_packet=SP,
                                queue_num=0,
                            )
                            dma_target = None
                        ioff += ni // 16

                        t = ncall
                        ncall += 1
                        for j in range(lo, hi):
                            w = int(cwin[j])
                            g = int(cgrp[j])
                            first = j == 0 or cwin[j - 1] != w
                            last = j == C - 1 or cwin[j + 1] != w
                            oh = ohp.tile([128, W], BF16, tag="oh")
                            nc.vector.tensor_scalar(
                                out=oh[:],
                                in0=iota_sb[:],
                                scalar1=dr_sb[:, j : j + 1],
                                scalar2=None,
                                op0=OP.is_equal,
                            )
                            if first:
                                wps = psW.tile(
                                    [FP, W], F32, space="PSUM", tag="wps"
                                )
                            lhs = mv[:, j - lo, g * FP : (g + 1) * FP]
                            mm = nc.tensor.matmul(
                                out=wps[:],
                                lhsT=lhs,
                                rhs=oh[:],
                                start=first,
                                stop=last,
                            )
                            if j == lo and dma_target is not None:
                                mm.wait_op(dma_sem, dma_target, "sem-ge")
                            if last:
                                gwin = gwp.tile([128, W], BF16, tag="gwin")
                                nc.scalar.copy(
                                    out=gwin[:FP, :],
                                    in_=wps[:],
                                )

                        ps = psC.tile([128, 512], F32, space="PSUM", tag="cps")
                        nc.tensor.matmul(
                            out=ps[:],
                            lhsT=wr_l,
                            rhs=hin[:, t * 512 : (t + 1) * 512],
                            start=True,
                            stop=False,
                        )
                        nc.tensor.matmul(
                            out=ps[:],
                            lhsT=wn_l,
                            rhs=gwin[:FP, :],
                            start=False,
                            stop=True,
                        )
                        sq = qp.tile([128, 512], F32, tag="sq")
                        nc.scalar.activation(
                            out=sq[:], in_=ps[:], func=AF.Square,
                            accum_out=s2[:, t : t + 1],
                        )
                        nc.scalar.activation(
                            out=h_raw[:, t * 512 : (t + 1) * 512], in_=ps[:],
                            func=AF.Copy, accum_out=s1[:, t : t + 1],
                        )

                    # BN stats combine + AllReduce
                    stats = sp.tile([128, 2], F32, tag="stats")
                    nc.vector.tensor_reduce(
                        out=stats[:, 0:1], in_=s1[:],
                        axis=mybir.AxisListType.X, op=OP.add,
                    )
                    nc.vector.tensor_reduce(
                        out=stats[:, 1:2], in_=s2[:],
                        axis=mybir.AxisListType.X, op=OP.add,
                    )
                    nc.sync.dma_start(out=bn_in[l], in_=stats[:])
                    nc.gpsimd.collective_compute(
                        "AllReduce",
                        OP.add,
                        replica_groups=rg,
                        ins=[bn_in[l].opt()],
                        outs=[bn_out[l].opt()],
                    )
                    gstats = sp.tile([128, 2], F32, tag="gstats")
                    nc.sync.dma_start(out=gstats[:], in_=bn_out[l])

                    pr = sp.tile([128, 6], F32, tag="bnpar")
                    nc.vector.tensor_scalar_mul(pr[:, 0:1], gstats[:, 0:1], 1.0 / N)
                    nc.vector.tensor_scalar_mul(pr[:, 1:2], gstats[:, 1:2], 1.0 / N)
                    nc.vector.tensor_tensor(
                        out=pr[:, 2:3], in0=pr[:, 0:1], in1=pr[:, 0:1], op=OP.mult
                    )
                    nc.vector.tensor_tensor(
                        out=pr[:, 1:2], in0=pr[:, 1:2], in1=pr[:, 2:3],
                        op=OP.subtract,
                    )
                    nc.vector.tensor_scalar_add(pr[:, 1:2], pr[:, 1:2], EPS)
                    nc.scalar.sqrt(out=pr[:, 2:3], in_=pr[:, 1:2])
                    nc.vector.reciprocal(out=pr[:, 3:4], in_=pr[:, 2:3])
                    nc.vector.tensor_tensor(
                        out=pr[:, 3:4], in0=pr[:, 3:4],
                        in1=bng_sb[:, l : l + 1], op=OP.mult,
                    )
                    nc.vector.tensor_tensor(
                        out=pr[:, 0:1], in0=pr[:, 0:1],
                        in1=cb_sb[:, l : l + 1], op=OP.add,
                    )
                    nc.vector.tensor_tensor(
                        out=pr[:, 4:5], in0=pr[:, 0:1], in1=pr[:, 3:4], op=OP.mult
                    )
                    nc.vector.tensor_tensor(
                        out=pr[:, 5:6], in0=bnb_sb[:, l : l + 1],
                        in1=pr[:, 4:5], op=OP.subtract,
                    )

                    # h = relu(h_raw * scale + shift)
                    for t in range(4):
                        nc.scalar.activation(
                            out=h_bf[:, t * 2048 : (t + 1) * 2048],
                            in_=h_raw[:, t * 2048 : (t + 1) * 2048],
                            func=AF.Relu,
                            bias=pr[:, 5:6],
                            scale=pr[:, 3:4],
                        )

                    if l < 2:
                        # node-major h -> DRAM -> AllGather (next layer table)
                        hNv = hN.rearrange("(b p) f -> p b f", p=128)
                        for w in range(NT):
                            tp = psT.tile([128, 128], BF16, space="PSUM", tag="tp")
                            nc.tensor.transpose(
                                out=tp[:],
                                in_=h_bf[:, w * 128 : (w + 1) * 128],
                                identity=ident_sb[:],
                            )
                            ts = gwp.tile([128, 128], BF16, tag="tstage")
                            nc.vector.tensor_copy(out=ts[:], in_=tp[:])
                            nc.sync.dma_start(out=hNv[:, w, :], in_=ts[:])
                        nc.gpsimd.collective_compute(
                            "AllGather",
                            OP.bypass,
                            replica_groups=rg,
                            ins=[hN.opt()],
                            outs=[hfull.opt()],
                        )

            # ----- heads (identical to v1 baseline)
            with (
                tc.tile_pool(name="hw", bufs=2) as hwp,
                tc.tile_pool(name="hsb", bufs=3) as hsb,
                tc.tile_pool(name="hps", bufs=2, space="PSUM") as hps,
                tc.tile_pool(name="hcst", bufs=1) as hc,
            ):
                pool = hc.tile([128, GPC], F32)
                nc.vector.tensor_reduce(
                    out=pool[:],
                    in_=h_bf[:].rearrange("p (g n) -> p g n", g=GPC),
                    axis=mybir.AxisListType.X,
                    op=OP.add,
                )
                gw = {}
                for name, apw, shape in (
                    ("gsw1", gsw1, [128, 128]),
                    ("gsw2", gsw2, [128, 128]),
                    ("ghw1", ghw1, [128, 128]),
                    ("ghw2", ghw2, [128, 64]),
                    ("ghw3", ghw3, [64, 2]),
                ):
                    t = hc.tile(shape, BF16, tag=name)
                    nc.sync.dma_start(out=t[:], in_=apw)
                    gw[name] = t
                gb = {}
                for name, apb, p in (
                    ("gsb1", gsb1, 128),
                    ("gsb2", gsb2, 128),
                    ("ghb1", ghb1, 128),
                    ("ghb2", ghb2, 64),
                    ("ghb3", ghb3, 2),
                ):
                    t = hc.tile([p, 1], F32, tag=name)
                    nc.sync.dma_start(out=t[:], in_=apb)
                    gb[name] = t

                g0 = hsb.tile([128, GPC], BF16, tag="g0")
                nc.scalar.activation(
                    out=g0[:], in_=pool[:], func=AF.Relu, scale=1.0 / NPG
                )
                gp1 = hps.tile([128, GPC], F32, space="PSUM", tag="gps")
                nc.tensor.matmul(
                    out=gp1[:], lhsT=gw["gsw1"][:], rhs=g0[:], start=True, stop=True
                )
                g1 = hsb.tile([128, GPC], BF16, tag="g1")
                nc.vector.tensor_scalar_add(g1[:], gp1[:], gb["gsb1"][:])
                gp2 = hps.tile([128, GPC], F32, space="PSUM", tag="gps")
                nc.tensor.matmul(
                    out=gp2[:], lhsT=gw["gsw2"][:], rhs=g1[:], start=True, stop=True
                )
                g2 = hsb.tile([128, GPC], BF16, tag="g2")
                nc.scalar.activation(
                    out=g2[:], in_=gp2[:], func=AF.Relu, bias=gb["gsb2"][:]
                )
                gp3 = hps.tile([128, GPC], F32, space="PSUM", tag="gps")
                nc.tensor.matmul(
                    out=gp3[:], lhsT=gw["ghw1"][:], rhs=g2[:], start=True, stop=True
                )
                g3 = hsb.tile([128, GPC], BF16, tag="g3")
                nc.scalar.activation(
                    out=g3[:], in_=gp3[:], func=AF.Relu, bias=gb["ghb1"][:]
                )
                gp4 = hps.tile([64, GPC], F32, space="PSUM", tag="gps")
                nc.tensor.matmul(
                    out=gp4[:], lhsT=gw["ghw2"][:], rhs=g3[:], start=True, stop=True
                )
                g4 = hsb.tile([64, GPC], BF16, tag="g4")
                nc.scalar.activation(
                    out=g4[:], in_=gp4[:], func=AF.Relu, bias=gb["ghb2"][:]
                )
                gp5 = hps.tile([2, GPC], F32, space="PSUM", tag="gps")
                nc.tensor.matmul(
                    out=gp5[:], lhsT=gw["ghw3"][:], rhs=g4[:], start=True, stop=True
                )
                gout = hsb.tile([2, GPC], F32, tag="gout")
                nc.vector.tensor_scalar_add(gout[:], gp5[:], gb["ghb3"][:])
                nc.sync.dma_start(out=outg, in_=gout[:])

                nb1 = hc.tile([128, 128], F32, tag="nb1")
                nc.sync.dma_start(out=nb1[:], in_=nhb1)
                nb2 = hc.tile([64, 128], F32, tag="nb2")
                nc.sync.dma_start(out=nb2[:], in_=nhb2)
                nb3 = hc.tile([1, 128], F32, tag="nb3")
                nc.sync.dma_start(out=nb3[:], in_=nhb3)
                w3 = hc.tile([64, 128], BF16, tag="w3")
                nc.sync.dma_start(out=w3[:], in_=nhw3)
                out_n = hc.tile([1, NPG * GPC], F32, tag="out_n")

                PCHUNK = 16
                for pc in range(NPG // PCHUNK):
                    w1 = hwp.tile([128, PCHUNK * 128], BF16, tag="w1")
                    nc.sync.dma_start(
                        out=w1[:],
                        in_=nhw1[:, pc * PCHUNK * 128 : (pc + 1) * PCHUNK * 128],
                    )
                    w2 = hwp.tile([128, PCHUNK * 64], BF16, tag="w2")
                    nc.sync.dma_start(
                        out=w2[:],
                        in_=nhw2[:, pc * PCHUNK * 64 : (pc + 1) * PCHUNK * 64],
                    )
                    for pi in range(PCHUNK):
                        p = pc * PCHUNK + pi
                        zp1 = hps.tile([128, GPC], F32, space="PSUM", tag="zp1")
                        nc.tensor.matmul(
                            out=zp1[:],
                            lhsT=w1[:, pi * 128 : (pi + 1) * 128],
                            rhs=h_bf[:, p :: NPG],
                            start=True,
                            stop=True,
                        )
                        z1 = hsb.tile([128, GPC], BF16, tag="z1")
                        nc.scalar.activation(
                            out=z1[:], in_=zp1[:], func=AF.Relu,
                            bias=nb1[:, p : p + 1],
                        )
                        zp2 = hps.tile([64, GPC], F32, space="PSUM", tag="zp2")
                        nc.tensor.matmul(
                            out=zp2[:],
                            lhsT=w2[:, pi * 64 : (pi + 1) * 64],
                            rhs=z1[:],
                            start=True,
                            stop=True,
                        )
                        z2 = hsb.tile([64, GPC], BF16, tag="z2")
                        nc.scalar.activation(
                            out=z2[:], in_=zp2[:], func=AF.Relu,
                            bias=nb2[:, p : p + 1],
                        )
                        zp3 = hps.tile([1, GPC], F32, space="PSUM", tag="zp3")
                        nc.tensor.matmul(
                            out=zp3[:],
                            lhsT=w3[:, p : p + 1],
                            rhs=z2[:],
                            start=True,
                            stop=True,
                        )
                        nc.vector.tensor_scalar_add(
                            out_n[:, p * GPC : (p + 1) * GPC],
                            zp3[:],
                            nb3[:, p : p + 1],
                        )
                nc.sync.dma_start(
                    out=outn.rearrange("(o p) g -> o (p g)", o=1), in_=out_n[:]
                )

    nc.compile()


# ------------------------------------------------------------------- driver

def _prep_inputs(inputs):
    f32 = lambda k: np.asarray(inputs[k], np.float32)
    bf16 = lambda a: np.ascontiguousarray(a).astype(NBF)

    edge_index = np.asarray(inputs["edge_index"], np.int64)
    plan12 = _build_plan(edge_index, 2)

    x = f32("x")
    iota = np.tile(np.arange(W, dtype=np.float32), (128, 1))

    shared = {
        "xq": bf16(np.concatenate(
            [x.reshape(N // 2, 64), np.zeros((N // 2, 64), np.float32)],
            axis=1)),
        "iota": iota.astype(np.float16),
        "ident": bf16(np.eye(128, dtype=np.float32)),
        "wn0": bf16(f32("conv0_wn")),
        "wr0": bf16(f32("conv0_wr")),
        "wn12": bf16(f32("convs_wn")),
        "wr12": bf16(f32("convs_wr")),
        "cb": np.stack(
            [f32("conv0_b"), f32("convs_b")[0], f32("convs_b")[1]], axis=1
        ).copy(),
        "bng": np.stack(
            [f32("bn0_g"), f32("bns_g")[0], f32("bns_g")[1]], axis=1
        ).copy(),
        "bnb": np.stack(
            [f32("bn0_b"), f32("bns_b")[0], f32("bns_b")[1]], axis=1
        ).copy(),
        "gsw1": bf16(f32("gs_w1")),
        "gsw2": bf16(f32("gs_w2")),
        "ghw1": bf16(f32("gh_w1")),
        "ghw2": bf16(f32("gh_w2")),
        "ghw3": bf16(f32("gh_w3")),
        "gsb1": f32("gs_b1").reshape(128, 1).copy(),
        "gsb2": f32("gs_b2").reshape(128, 1).copy(),
        "ghb1": f32("gh_b1").reshape(128, 1).copy(),
        "ghb2": f32("gh_b2").reshape(64, 1).copy(),
        "ghb3": f32("gh_b3").reshape(2, 1).copy(),
        "nhw1": bf16(f32("nh_w1").transpose(1, 0, 2).reshape(128, 128 * 128)),
        "nhw2": bf16(f32("nh_w2").transpose(1, 0, 2).reshape(128, 128 * 64)),
        "nhw3": bf16(f32("nh_w3")[:, :, 0].T),
        "nhb1": f32("nh_b1").T.copy(),
        "nhb2": f32("nh_b2").T.copy(),
        "nhb3": f32("nh_b3").T.copy(),
    }

    in_maps = []
    for c in range(NC):
        in_maps.append(
            dict(
                shared,
                xloc=bf16(x[c * NPC : (c + 1) * NPC].T),
                idx12=_wrap_calls(plan12[3][c], plan12[2]),
                dr12=plan12[4][c].T.astype(np.float32).copy(),
            )
        )
    return plan12, in_maps


def _numpy_fallback(inputs):
    f = lambda k: np.asarray(inputs[k], np.float32)
    x = f("x")
    src, dst = np.asarray(inputs["edge_index"], np.int64)
    batch = np.asarray(inputs["batch"], np.int64)

    def gconv(h, wr, wn, b):
        y = h @ wn
        agg = np.zeros_like(h @ wr)
        np.add.at(agg, dst, y[src])
        return h @ wr + agg + b

    def bn(h, g, bt):
        m = h.mean(0)
        v = h.var(0)
        return (h - m) / np.sqrt(v + EPS) * g + bt

    h = np.maximum(bn(gconv(x, f("conv0_wr"), f("conv0_wn"), f("conv0_b")),
                      f("bn0_g"), f("bn0_b")), 0)
    for i in range(2):
        h = np.maximum(
            bn(gconv(h, f("convs_wr")[i], f("convs_wn")[i], f("convs_b")[i]),
               f("bns_g")[i], f("bns_b")[i]), 0)
    counts = np.bincount(batch, minlength=B).astype(np.float32)
    xg = np.zeros((B, H), np.float32)
    np.add.at(xg, batch, h)
    xg /= counts[:, None]
    g = np.maximum(xg, 0)
    g = g @ f("gs_w1") + f("gs_b1")
    g = np.maximum(g @ f("gs_w2") + f("gs_b2"), 0)
    g = np.maximum(g @ f("gh_w1") + f("gh_b1"), 0)
    g = np.maximum(g @ f("gh_w2") + f("gh_b2"), 0)
    g = g @ f("gh_w3") + f("gh_b3")
    xn = h.reshape(B, NPG, H)
    z = np.maximum(np.einsum("bnf,nfh->bnh", xn, f("nh_w1")) + f("nh_b1"), 0)
    z = np.maximum(np.einsum("bnh,nhk->bnk", z, f("nh_w2")) + f("nh_b2"), 0)
    z = np.einsum("bnk,nko->bno", z, f("nh_w3")) + f("nh_b3")
    return np.concatenate([g, z[:, :, 0]], axis=1).astype(np.float32)


def _run(inputs, trace=False, trace_kwargs=None):
    batch = np.asarray(inputs["batch"], np.int64)
    if not (
        np.array_equal(batch, np.arange(N, dtype=np.int64) // NPG)
        and np.asarray(inputs["x"]).shape == (N, 32)
        and np.asarray(inputs["edge_index"]).shape == (2, E)
    ):
        return _numpy_fallback(inputs), None

    plan12, in_maps = _prep_inputs(inputs)
    nc = bacc.Bacc(
        "TRN2", target_bir_lowering=False, debug=False, num_devices=NC,
        dynamic_dma_scratch_size=32768,
    )
    _build(nc, plan12)
    r = run_bass_kernel_spmd(
        nc, in_maps, list(range(NC)), trace=trace, **(trace_kwargs or {})
    )
    out = np.zeros((B, 2 + NPG), np.float32)
    for c in range(NC):
        out[c * GPC : (c + 1) * GPC, 0:2] = r.results[c]["outg"].T
        out[c * GPC : (c + 1) * GPC, 2:] = r.results[c]["outn"].T
    return out, r


def kernel(**inputs):
    out, _ = _run(inputs)
    return out

